# revision 1
# baseline (speedup 1.0000x reference)
"""BlockSparseMLA Trainium2 kernel.

Sharding: 8 cores = 2 batches x 4 seq-quarters. Each core computes all 16
heads for its 512 queries: q projection, latent/kv up-projection at the 256
selected key positions, RoPE, sparse causal attention over the selected
keys, and the full w_out projection for its rows. Host does block scoring /
top-k, gathers selected positions, builds the causal mask over selected
keys, and patches the degenerate all-masked rows (uniform attention over
all positions) with a host-computed rank-1 fallback.

Device layouts are all "transposed" (feature dim on partitions) so no PE
transposes are needed anywhere:
  qT [c=16h*64, s]  kT [c, keys]  v [keys, c]  scoresT/expT [keys, s]
  yT [c, s]  out [s, dout]
Softmax skips max-subtraction (|scores| is small; masked lanes multiply
exp by 0), Z comes from a ones-column matmul, empty rows survive via
max(Z, 1e-30) and are overwritten on the host.
"""

import sys

import numpy as np

sys.path.insert(0, "/opt/trn_rl_repo")

from contextlib import ExitStack

import concourse.bacc as bacc
import concourse.bass as bass
import concourse.mybir as mybir
import concourse.tile as tile

B, S, D = 2, 2048, 1024
H, HD, R = 16, 64, 128
BLOCK, TOPK = 64, 4
ROPE_BASE = 100000.0
SQ = S // 4
KEYS = TOPK * BLOCK  # 256
CK = D // 128  # c chunks (2 heads each)
DK = D // 128  # d chunks
F32 = mybir.dt.float32

USE_F32R = True  # feed matmuls as float32r (fp22 single-pass, 4x faster)


def _f32(a):
    return np.ascontiguousarray(a, dtype=np.float32)


def _wvup_zp(w_kv_up):
    """w_kv_up_v.T zero-padded so head h's 64 v-columns sit at
    cols h*128 + (h%2)*64 of a [R, 2048] matrix (other half zero).
    PV matmuls then write full [128, s] PSUM tiles at partition base 0."""
    wv = np.asarray(w_kv_up, np.float32)[D:].T  # [R, D]
    wz = np.zeros((R, 2 * D), np.float32)
    for h in range(H):
        c0 = h * 128 + (h % 2) * 64
        wz[:, c0 : c0 + 64] = wv[:, h * 64 : (h + 1) * 64]
    return wz


def _perm():
    """[128, 128] block-diag rotate-half permutation: out[p] = in[p^32
    within each 64-block] (symmetric). Used as matmul lhsT on the PE to
    produce the rotated copy without SBUF-to-SBUF shift DMAs."""
    P = np.zeros((128, 128), np.float32)
    for pp in range(128):
        blk, e = divmod(pp, 64)
        s = blk * 64 + (e + 32 if e < 32 else e - 32)
        P[s, pp] = 1.0
    return P


def _onesz():
    """[128, 256]: block hi occupies cols hi*128..hi*128+128 with ones in
    its own 64-row half, zeros elsewhere (Z-broadcast matmul lhsT)."""
    oz = np.zeros((128, 256), np.float32)
    oz[:, 0:64] = 1.0
    oz[:, 192:256] = 1.0
    return oz


def host_prep(x, w_q, w_kv_down, w_kv_up, w_out, w_scorer):
    """Returns (in_maps for 8 cores, qmin[B], fb_rows[B, D])."""
    x = np.asarray(x, dtype=np.float32)
    nb = S // BLOCK

    reps = x.reshape(B, nb, BLOCK, D).mean(axis=2)
    scores = reps @ np.asarray(w_scorer, np.float32)[0]
    top = np.argsort(-scores, axis=1, kind="stable")[:, :TOPK]
    sel_blocks = np.sort(top, axis=1)
    qmin = sel_blocks[:, 0] * BLOCK
    sel_pos = (
        sel_blocks[:, :, None] * BLOCK + np.arange(BLOCK)[None, None, :]
    ).reshape(B, KEYS)

    # RoPE tables (fp32, mirrors reference._rope_tables)
    half = np.arange(0, HD, 2, dtype=np.float32) / np.float32(HD)
    inv_freq = np.float32(1.0) / np.power(np.float32(ROPE_BASE), half)
    freqs = np.arange(S, dtype=np.float32)[:, None] * inv_freq[None, :]
    emb = np.concatenate([freqs, freqs], axis=1)  # [S, HD]
    cos = np.cos(emb).astype(np.float32)
    sin = np.sin(emb).astype(np.float32)
    sgn = np.where(np.arange(HD) < HD // 2, np.float32(-1.0), np.float32(1.0))
    sins = sin * sgn[None, :]  # signed sin for shift-based rotate_half

    # Fallback row for fully-masked queries: uniform attention over all S
    # positions -> mean(v) -> w_out.  (v = latent @ w_kv_up_v.T is linear.)
    latent_mean = x.mean(axis=1) @ np.asarray(w_kv_down, np.float32).T  # [B, R]
    v_mean = latent_mean @ np.asarray(w_kv_up, np.float32)[D:].T  # [B, D]
    fb_rows = v_mean @ np.asarray(w_out, np.float32).T  # [B, D]

    w_q = np.asarray(w_q, np.float32)
    w_kv_down = np.asarray(w_kv_down, np.float32)
    w_kv_up = np.asarray(w_kv_up, np.float32)
    w_out = np.asarray(w_out, np.float32)

    shared = {
        "wqT": _f32(w_q.T),
        "wkvdT": _f32(w_kv_down.T),
        "wkupT": _f32(w_kv_up[:D].T),
        "wvupT": _wvup_zp(w_kv_up),
        "onesz": _onesz(),
        "perm": _perm(),
        "woutT": _f32(w_out.T),
    }
    in_maps = []
    for c in range(8):
        b, sq = divmod(c, 4)
        s0 = sq * SQ
        m = dict(shared)
        m["xT"] = _f32(x[b, s0 : s0 + SQ].T)
        m["xselT"] = _f32(x[b, sel_pos[b]].T)
        m["cosq"] = _f32(np.tile(cos[s0 : s0 + SQ].T, (2, 1)))
        m["sinq"] = _f32(np.tile(sins[s0 : s0 + SQ].T, (2, 1)))
        m["cosk"] = _f32(np.tile(cos[sel_pos[b]].T, (2, 1)))
        m["sink"] = _f32(np.tile(sins[sel_pos[b]].T, (2, 1)))
        m["maskT"] = _f32(
            sel_pos[b][:, None] <= (s0 + np.arange(SQ))[None, :]
        )
        in_maps.append(m)
    return in_maps, qmin, fb_rows


def build_nc(use_f32r=USE_F32R):
    nc = bacc.Bacc("TRN2", target_bir_lowering=False)

    FD = mybir.dt.float32r if use_f32r else F32

    def mmc(ap):
        return ap

    xT = nc.dram_tensor("xT", [D, SQ], FD, kind="ExternalInput")
    xselT = nc.dram_tensor("xselT", [D, KEYS], FD, kind="ExternalInput")
    wqT = nc.dram_tensor("wqT", [D, D], FD, kind="ExternalInput")
    wkvdT = nc.dram_tensor("wkvdT", [D, R], FD, kind="ExternalInput")
    wkupT = nc.dram_tensor("wkupT", [R, D], FD, kind="ExternalInput")
    wvupT = nc.dram_tensor("wvupT", [R, 2 * D], FD, kind="ExternalInput")
    onesz = nc.dram_tensor("onesz", [128, 256], FD, kind="ExternalInput")
    perm = nc.dram_tensor("perm", [128, 128], FD, kind="ExternalInput")
    woutT = nc.dram_tensor("woutT", [D, D], FD, kind="ExternalInput")
    cosq = nc.dram_tensor("cosq", [128, SQ], F32, kind="ExternalInput")
    sinq = nc.dram_tensor("sinq", [128, SQ], F32, kind="ExternalInput")
    cosk = nc.dram_tensor("cosk", [128, KEYS], F32, kind="ExternalInput")
    sink = nc.dram_tensor("sink", [128, KEYS], F32, kind="ExternalInput")
    maskT = nc.dram_tensor("maskT", [KEYS, SQ], F32, kind="ExternalInput")
    out = nc.dram_tensor("out", [SQ, D], F32, kind="ExternalOutput")

    EXP = mybir.ActivationFunctionType.Exp

    with tile.TileContext(nc) as tc, ExitStack() as ctx:
        const = ctx.enter_context(tc.tile_pool(name="const", bufs=1))

        # ---- persistent inputs (small early-stage tensors first)
        xsel_sb = const.tile([128, DK, KEYS], FD, tag="xsel")
        nc.sync.dma_start(
            xsel_sb[:], xselT[:, :].rearrange("(k p) s -> p k s", p=128)
        )
        wkvd_sb = const.tile([128, DK, R], FD, tag="wkvd")
        nc.sync.dma_start(
            wkvd_sb[:], wkvdT[:, :].rearrange("(k p) r -> p k r", p=128)
        )
        wkup_sb = const.tile([128, D], FD, tag="wkup")
        nc.sync.dma_start(wkup_sb[:], wkupT[:, :])
        wvup_sb = const.tile([128, 2 * D], FD, tag="wvup")
        nc.sync.dma_start(wvup_sb[:, 0:D], wvupT[:, 0:D])
        nc.sync.dma_start(wvup_sb[:, D : 2 * D], wvupT[:, D : 2 * D])
        perm_sb = const.tile([128, 128], FD, tag="perm")
        nc.sync.dma_start(perm_sb[:], perm[:, :])
        cosk_sb = const.tile([128, KEYS], F32, tag="cosk")
        nc.sync.dma_start(cosk_sb[:], cosk[:, :])
        sink_sb = const.tile([128, KEYS], F32, tag="sink")
        nc.sync.dma_start(sink_sb[:], sink[:, :])
        onesz_sb = const.tile([128, 256], FD, tag="onesz")
        nc.sync.dma_start(onesz_sb[:], onesz[:, :])
        cosq_sb = const.tile([128, SQ], F32, tag="cosq")
        nc.sync.dma_start(cosq_sb[:], cosq[:, :])
        sinq_sb = const.tile([128, SQ], F32, tag="sinq")
        nc.sync.dma_start(sinq_sb[:], sinq[:, :])
        mask_sb = const.tile([128, 2, SQ], F32, tag="mask")
        nc.sync.dma_start(
            mask_sb[:], maskT[:, :].rearrange("(m p) s -> p m s", p=128)
        )

        # ---- results that span stages
        kT_sb = const.tile([128, CK, KEYS], FD, tag="kT")
        v_sb = const.tile([128, 2, 2 * D], FD, tag="v")
        qTr_sb = const.tile([128, CK, SQ], FD, tag="qTr")
        yT_sb = const.tile([128, CK, SQ], FD, tag="yT")

        # ================= stages A-D (xT/wq scoped: freed afterwards) ====
        with (
            tc.tile_pool(name="big_in", bufs=1) as big_in,
            tc.tile_pool(name="work", bufs=2) as work,
            tc.tile_pool(name="ps_e", bufs=2, space="PSUM") as ps_e,
            tc.tile_pool(name="ps_q", bufs=2, space="PSUM") as ps_q,
            tc.tile_pool(name="ps_r", bufs=2, space="PSUM") as ps_r,
        ):
            xT_sb = big_in.tile([128, DK, SQ], FD, tag="xT")
            for dk2 in range(0, DK, 2):
                nc.sync.dma_start(
                    xT_sb[:, dk2 : dk2 + 2, :],
                    xT[dk2 * 128 : (dk2 + 2) * 128, :].rearrange(
                        "(k p) s -> p k s", p=128
                    ),
                )
            wq_sb = big_in.tile([128, DK, D], FD, tag="wq")
            for dk2 in range(DK):
                nc.sync.dma_start(
                    wq_sb[:, dk2, :], wqT[dk2 * 128 : (dk2 + 1) * 128, :]
                )

            # ---- stage A: latentT at selected positions [R, KEYS]
            lat_ps = ps_e.tile([128, KEYS], F32, tag="early")
            for dk in range(DK):
                nc.tensor.matmul(
                    lat_ps[:],
                    mmc(wkvd_sb[:, dk, :]),
                    mmc(xsel_sb[:, dk, :]),
                    start=(dk == 0),
                    stop=(dk == DK - 1),
                )
            lat_sb = const.tile([128, KEYS], FD, tag="lat")
            nc.scalar.copy(lat_sb[:], lat_ps[:])

            # ---- stage B: kT chunks + RoPE -> kT_sb [c, keys]
            for ck in range(CK):
                k_ps = ps_e.tile([128, KEYS], F32, tag="early")
                nc.tensor.matmul(
                    k_ps[:],
                    mmc(wkup_sb[:, ck * 128 : (ck + 1) * 128]),
                    mmc(lat_sb[:]),
                    start=True,
                    stop=True,
                )
                k_raw = work.tile([128, KEYS], FD, tag="k_raw")
                nc.scalar.copy(k_raw[:], k_ps[:])
                k_rot = ps_r.tile([128, KEYS], F32, tag="rot")
                nc.tensor.matmul(
                    k_rot[:], mmc(perm_sb[:]), mmc(k_raw[:]), start=True, stop=True
                )
                kt1 = work.tile([128, KEYS], F32, tag="kt1")
                nc.gpsimd.tensor_mul(kt1[:], k_raw[:], cosk_sb[:])
                kt2 = work.tile([128, KEYS], F32, tag="kt2")
                nc.vector.tensor_mul(kt2[:], k_rot[:], sink_sb[:])
                nc.gpsimd.tensor_add(kT_sb[:, ck, :], kt1[:], kt2[:])

            # ---- stage C: v [keys, c] (zero-padded per head)
            for mk in range(2):
                for nh in range(4):
                    v_ps = ps_e.tile([128, 512], F32, tag="early")
                    nc.tensor.matmul(
                        v_ps[:],
                        mmc(lat_sb[:, mk * 128 : (mk + 1) * 128]),
                        mmc(wvup_sb[:, nh * 512 : (nh + 1) * 512]),
                        start=True,
                        stop=True,
                    )
                    if nh % 2 == 0:
                        nc.scalar.copy(v_sb[:, mk, nh * 512 : (nh + 1) * 512], v_ps[:])
                    else:
                        nc.vector.tensor_copy(
                            v_sb[:, mk, nh * 512 : (nh + 1) * 512], v_ps[:]
                        )

            # ---- stage D: qT chunks + RoPE -> qTr_sb [c, s]
            for ck in range(CK):
                q_ps = ps_q.tile([128, SQ], F32, tag="qT")
                for dk in range(DK):
                    nc.tensor.matmul(
                        q_ps[:],
                        mmc(wq_sb[:, dk, ck * 128 : (ck + 1) * 128]),
                        mmc(xT_sb[:, dk, :]),
                        start=(dk == 0),
                        stop=(dk == DK - 1),
                    )
                q_raw = work.tile([128, SQ], FD, tag="q_raw")
                nc.scalar.copy(q_raw[:], q_ps[:])
                q_rot = ps_r.tile([128, SQ], F32, tag="rot")
                nc.tensor.matmul(
                    q_rot[:], mmc(perm_sb[:]), mmc(q_raw[:]), start=True, stop=True
                )
                qt1 = work.tile([128, SQ], F32, tag="qt1")
                nc.gpsimd.tensor_mul(qt1[:], q_raw[:], cosq_sb[:])
                qt2 = work.tile([128, SQ], F32, tag="qt2")
                nc.vector.tensor_mul(qt2[:], q_rot[:], sinq_sb[:])
                nc.gpsimd.tensor_add(qTr_sb[:, ck, :], qt1[:], qt2[:])

        # ================= stage E =================
        with (
            tc.tile_pool(name="epool", bufs=6) as epool,
            tc.tile_pool(name="ework", bufs=3) as ework,
            tc.tile_pool(name="ps_sc", bufs=2, space="PSUM") as ps_sc,
            tc.tile_pool(name="ps_o", bufs=2, space="PSUM") as ps_o,
            tc.tile_pool(name="ps_z", bufs=2, space="PSUM") as ps_z,
        ):
            for p in range(CK):
                z_ps = ps_z.tile([128, SQ], F32, tag="z")
                outT2 = ps_o.tile([128, SQ], F32, tag="outT")
                for hi in range(2):
                    h = 2 * p + hi
                    pb = hi * 64
                    # both key chunks of this head in one 2-bank psum tile
                    sc_ps = ps_sc.tile([128, 2, SQ], F32, tag="sc")
                    for mk in range(2):
                        nc.tensor.matmul(
                            sc_ps[:, mk, :],
                            mmc(kT_sb[pb : pb + 64, p, mk * 128 : (mk + 1) * 128]),
                            mmc(qTr_sb[pb : pb + 64, p, :]),
                            start=True,
                            stop=True,
                        )
                    expU = epool.tile([128, 2, SQ], F32, tag="expU")
                    nc.scalar.activation(
                        expU[:].rearrange("p m s -> p (m s)"),
                        sc_ps[:].rearrange("p m s -> p (m s)"),
                        EXP,
                        scale=0.125,
                    )
                    expT = epool.tile([128, 2, SQ], FD, tag="expT")
                    if hi == 0:
                        nc.gpsimd.tensor_mul(
                            expT[:].rearrange("p m s -> p (m s)"),
                            expU[:].rearrange("p m s -> p (m s)"),
                            mask_sb[:].rearrange("p m s -> p (m s)"),
                        )
                    else:
                        nc.vector.tensor_mul(
                            expT[:].rearrange("p m s -> p (m s)"),
                            expU[:].rearrange("p m s -> p (m s)"),
                            mask_sb[:].rearrange("p m s -> p (m s)"),
                        )
                    for mk in range(2):
                        nc.tensor.matmul(
                            z_ps[:],
                            mmc(onesz_sb[:, hi * 128 : (hi + 1) * 128]),
                            mmc(expT[:, mk, :]),
                            start=(hi == 0 and mk == 0),
                            stop=(hi == 1 and mk == 1),
                        )
                        nc.tensor.matmul(
                            outT2[:],
                            mmc(v_sb[:, mk, h * 128 : (h + 1) * 128]),
                            mmc(expT[:, mk, :]),
                            start=(hi == 0 and mk == 0),
                            stop=(hi == 1 and mk == 1),
                        )
                zc = ework.tile([128, SQ], F32, tag="zc")
                nc.vector.tensor_scalar_max(zc[:], z_ps[:], 1e-30)
                zr = ework.tile([128, SQ], F32, tag="zr")
                nc.vector.reciprocal(zr[:], zc[:])
                nc.vector.tensor_mul(yT_sb[:, p, :], outT2[:], zr[:])

        # ================= stage F: out = yT.T @ woutT ====================
        with (
            tc.tile_pool(name="ps_w", bufs=1, space="PSUM") as ps_w,
            tc.tile_pool(name="wst", bufs=4) as wst,
            tc.tile_pool(name="ost", bufs=2) as ost,
        ):
            outps = [
                ps_w.tile([128, 512], F32, tag=f"w{i}", name=f"outps{i}")
                for i in range(8)
            ]
            for ck in range(CK):
                wo = wst.tile([128, D], FD, tag="wo")
                nc.sync.dma_start(wo[:], woutT[ck * 128 : (ck + 1) * 128, :])
                for st in range(4):
                    for dh in range(2):
                        nc.tensor.matmul(
                            outps[st * 2 + dh][:],
                            mmc(yT_sb[:, ck, st * 128 : (st + 1) * 128]),
                            mmc(wo[:, dh * 512 : (dh + 1) * 512]),
                            start=(ck == 0),
                            stop=(ck == CK - 1),
                        )
            for st in range(4):
                o_sb = ost.tile([128, D], F32, tag="osb")
                nc.scalar.copy(o_sb[:, 0:512], outps[st * 2][:])
                nc.vector.tensor_copy(o_sb[:, 512:1024], outps[st * 2 + 1][:])
                nc.sync.dma_start(out[st * 128 : (st + 1) * 128, :], o_sb[:])

    nc.compile()
    return nc


_NC_CACHE = {}


def _get_nc():
    key = USE_F32R
    if key not in _NC_CACHE:
        _NC_CACHE[key] = build_nc(key)
    return _NC_CACHE[key]


TRACE = False  # set by test harness to capture an NTFF profile
LAST_RESULTS = None


def kernel(x, w_q, w_kv_down, w_kv_up, w_out, w_scorer):
    global LAST_RESULTS
    from concourse.bass_utils import run_bass_kernel_spmd

    in_maps, qmin, fb_rows = host_prep(x, w_q, w_kv_down, w_kv_up, w_out, w_scorer)
    nc = _get_nc()
    res = run_bass_kernel_spmd(nc, in_maps, core_ids=list(range(8)), trace=TRACE)
    LAST_RESULTS = res
    out = np.empty((B, S, D), np.float32)
    for c in range(8):
        b, sq = divmod(c, 4)
        out[b, sq * SQ : (sq + 1) * SQ] = res.results[c]["out"]
    for b in range(B):
        if qmin[b] > 0:
            out[b, : qmin[b]] = fb_rows[b]
    return out



# revision 6
# speedup vs baseline: 1.3001x; 1.3001x over previous
"""BlockSparseMLA Trainium2 kernel (v2 — bf16 streaming redesign).

Sharding: 8 cores = 2 batches x 4 seq-quarters; each core runs all 16 heads
for its 512 queries over the 256 selected key positions. Host does block
scoring / top-k / gather / mask build and patches fully-masked rows.

v2 design notes (vs v1 baseline at 151us):
  * all matmul inputs bf16 (halves DMA, 1 cyc/row PE) — rel err stays ~1e-3
  * all DRAM inputs pre-packed host-side as [128, N] partition-major blocks
    (contiguous 2-16KB per-partition lines -> large DMA packets)
  * PE warmup matmuls during the initial DMA wait so the tensor engine's
    DVFS ramp (0.65 -> 1.2 -> 2.4 GHz after ~3us continuous busy) completes
    before real work, and stages are ordered to keep PE busy end-to-end
  * q-projection first (ck-outer, dk accumulation) streaming against the
    per-ck weight DMAs; k rotation folded into a second up-projection weight
    (wkupP) so RoPE needs no PE->copy->PE hop for k
  * softmax Z fused into the PV matmul: v lhsT carries [v_h | ones] so psum
    rows 0-63 = y_h, rows 64-127 = Z_h; no separate Z matmuls
  * reciprocal via reciprocal_approx_fast (5x faster than reciprocal)
  * stage F st-outer accumulation -> output DMA overlaps remaining compute
"""

import sys

import numpy as np

sys.path.insert(0, "/opt/trn_rl_repo")

from contextlib import ExitStack

import ml_dtypes

import concourse.bacc as bacc
import concourse.bass as bass
import concourse.mybir as mybir
import concourse.tile as tile

B, S, D = 2, 2048, 1024
H, HD, R = 16, 64, 128
BLOCK, TOPK = 64, 4
ROPE_BASE = 100000.0
SQ = S // 4
KEYS = TOPK * BLOCK  # 256
CK = D // 128  # output c chunks (2 heads each)
DK = D // 128  # input d chunks
F32 = mybir.dt.float32
F32R = mybir.dt.float32r
BF16 = mybir.dt.bfloat16
NPBF = ml_dtypes.bfloat16

N_WARMUP = 10  # PE warmup matmuls (512 cols each) during initial DMA wait


def _bf(a):
    return np.ascontiguousarray(np.asarray(a, np.float32).astype(NPBF))


def _f32(a):
    return np.ascontiguousarray(a, dtype=np.float32)


def _perm():
    """[128, 128] block-diag rotate-half permutation: out[c] = in[c^32
    within each 64-block]. Used as matmul lhsT for the q rotation."""
    P = np.zeros((128, 128), np.float32)
    for pp in range(128):
        blk, e = divmod(pp, 64)
        s = blk * 64 + (e + 32 if e < 32 else e - 32)
        P[s, pp] = 1.0
    return P


def host_prep(x, w_q, w_kv_down, w_kv_up, w_out, w_scorer):
    """Returns (in_maps for 8 cores, qmin[B], fb_rows[B, D])."""
    x = np.asarray(x, dtype=np.float32)
    nb = S // BLOCK

    reps = x.reshape(B, nb, BLOCK, D).mean(axis=2)
    scores = reps @ np.asarray(w_scorer, np.float32)[0]
    top = np.argsort(-scores, axis=1, kind="stable")[:, :TOPK]
    sel_blocks = np.sort(top, axis=1)
    qmin = sel_blocks[:, 0] * BLOCK
    sel_pos = (
        sel_blocks[:, :, None] * BLOCK + np.arange(BLOCK)[None, None, :]
    ).reshape(B, KEYS)

    # RoPE tables (fp32, mirrors reference._rope_tables)
    half = np.arange(0, HD, 2, dtype=np.float32) / np.float32(HD)
    inv_freq = np.float32(1.0) / np.power(np.float32(ROPE_BASE), half)
    freqs = np.arange(S, dtype=np.float32)[:, None] * inv_freq[None, :]
    emb = np.concatenate([freqs, freqs], axis=1)  # [S, HD]
    cos = np.cos(emb).astype(np.float32)
    sin = np.sin(emb).astype(np.float32)
    sgn = np.where(np.arange(HD) < HD // 2, np.float32(-1.0), np.float32(1.0))
    sins = sin * sgn[None, :]  # signed sin for swap-based rotate_half

    # Fallback row for fully-masked queries: uniform attention over all S
    # positions -> mean(v) -> w_out.
    latent_mean = x.mean(axis=1) @ np.asarray(w_kv_down, np.float32).T  # [B, R]
    v_mean = latent_mean @ np.asarray(w_kv_up, np.float32)[D:].T  # [B, D]
    fb_rows = v_mean @ np.asarray(w_out, np.float32).T  # [B, D]

    w_q = np.asarray(w_q, np.float32)
    w_kv_down = np.asarray(w_kv_down, np.float32)
    w_kv_up = np.asarray(w_kv_up, np.float32)
    w_out = np.asarray(w_out, np.float32)

    # --- shared weight blocks, pre-packed [128, ...] partition-major ---
    # wq_ck[p, dk, c'] = w_q[ck*128+c', dk*128+p]
    wq4 = w_q.reshape(CK, 128, DK, 128).transpose(3, 0, 2, 1)  # [p,ck,dk,c']
    wq_cks = [_bf(wq4[:, ck].reshape(128, DK * 128)) for ck in range(CK)]
    # wkvd[p, dk, r] = w_kv_down[r, dk*128+p]
    wkvd = _bf(w_kv_down.reshape(R, DK, 128).transpose(2, 1, 0).reshape(128, -1))
    # wkup[r, ck, c] = w_kv_up[ck*128+c, r] (k half)
    wkupT = w_kv_up[:D].T  # [R, D]
    wkup = _bf(wkupT.reshape(128, CK, 128).reshape(128, -1))
    # wkupP: columns rotate-half permuted within each head's 64-block
    cidx = np.arange(D)
    e = cidx % 64
    pidx = (cidx // 64) * 64 + np.where(e < 32, e + 32, e - 32)
    wkupP = _bf(wkupT[:, pidx].reshape(128, -1))
    # wvup[r, j] = w_kv_up[D + j, r] (v half, head-major j)
    wvup = _bf(w_kv_up[D:].T.reshape(128, -1))
    # wout[p, ck, j] = w_out[j, ck*128+p]
    wout = _bf(w_out.T.reshape(CK, 128, D).transpose(1, 0, 2).reshape(128, -1))
    perm = _f32(_perm())

    shared = {}
    for ck in range(CK):
        shared[f"wq{ck}"] = wq_cks[ck]
    shared.update(
        wkvd=wkvd, wkup=wkup, wkupP=wkupP, wvup=wvup, wout=wout, perm=perm
    )

    in_maps = []
    for c in range(8):
        b, sq = divmod(c, 4)
        s0 = sq * SQ
        m = dict(shared)
        # xT[p, dk, s] = x[b, s0+s, dk*128+p]
        m["xT"] = _bf(
            x[b, s0 : s0 + SQ].reshape(SQ, DK, 128).transpose(2, 1, 0).reshape(128, -1)
        )
        m["xsel"] = _bf(
            x[b, sel_pos[b]].reshape(KEYS, DK, 128).transpose(2, 1, 0).reshape(128, -1)
        )
        m["cosq"] = _f32(np.tile(cos[s0 : s0 + SQ].T, (2, 1)))
        m["sinq"] = _f32(np.tile(sins[s0 : s0 + SQ].T, (2, 1)))
        m["cosk"] = _f32(np.tile(cos[sel_pos[b]].T, (2, 1)))
        m["sink"] = _f32(np.tile(sins[sel_pos[b]].T, (2, 1)))
        # mask[p, mk, s] = sel_pos[mk*128+p] <= s0+s
        mk = (
            sel_pos[b][:, None] <= (s0 + np.arange(SQ))[None, :]
        ).reshape(2, 128, SQ).transpose(1, 0, 2)
        m["maskT"] = _bf(mk.reshape(128, -1))
        in_maps.append(m)
    return in_maps, qmin, fb_rows


def build_nc():
    nc = bacc.Bacc("TRN2", target_bir_lowering=False)

    xT = nc.dram_tensor("xT", [128, DK * SQ], BF16, kind="ExternalInput")
    xsel = nc.dram_tensor("xsel", [128, DK * KEYS], BF16, kind="ExternalInput")
    wq_d = [
        nc.dram_tensor(f"wq{ck}", [128, DK * 128], BF16, kind="ExternalInput")
        for ck in range(CK)
    ]
    wkvd = nc.dram_tensor("wkvd", [128, DK * R], BF16, kind="ExternalInput")
    wkup = nc.dram_tensor("wkup", [128, D], BF16, kind="ExternalInput")
    wkupP = nc.dram_tensor("wkupP", [128, D], BF16, kind="ExternalInput")
    wvup = nc.dram_tensor("wvup", [128, D], BF16, kind="ExternalInput")
    wout = nc.dram_tensor("wout", [128, CK * D], BF16, kind="ExternalInput")
    perm = nc.dram_tensor("perm", [128, 128], F32R, kind="ExternalInput")
    cosq = nc.dram_tensor("cosq", [128, SQ], F32, kind="ExternalInput")
    sinq = nc.dram_tensor("sinq", [128, SQ], F32, kind="ExternalInput")
    cosk = nc.dram_tensor("cosk", [128, KEYS], F32, kind="ExternalInput")
    sink = nc.dram_tensor("sink", [128, KEYS], F32, kind="ExternalInput")
    maskT = nc.dram_tensor("maskT", [128, 2 * SQ], BF16, kind="ExternalInput")
    out = nc.dram_tensor("out", [SQ, D], F32, kind="ExternalOutput")

    EXP = mybir.ActivationFunctionType.Exp

    with tile.TileContext(nc) as tc, ExitStack() as ctx:
        const = ctx.enter_context(tc.tile_pool(name="const", bufs=1))

        # ---- DMA issue order == criticality order ----
        xT_sb = const.tile([128, DK, SQ], BF16, tag="xT")
        nc.sync.dma_start(xT_sb[:], xT[:, :].rearrange("p (k s) -> p k s", k=DK))
        wq_sb = []
        for ck in range(CK):
            t = const.tile([128, DK, 128], BF16, tag=f"wq{ck}")
            nc.sync.dma_start(
                t[:], wq_d[ck][:, :].rearrange("p (k c) -> p k c", k=DK)
            )
            wq_sb.append(t)
            if ck == 0:
                perm_sb = const.tile([128, 128], F32R, tag="perm")
                nc.sync.dma_start(perm_sb[:], perm[:, :])
        xsel_sb = const.tile([128, DK, KEYS], BF16, tag="xsel")
        nc.sync.dma_start(
            xsel_sb[:], xsel[:, :].rearrange("p (k s) -> p k s", k=DK)
        )
        wkvd_sb = const.tile([128, DK, R], BF16, tag="wkvd")
        nc.sync.dma_start(
            wkvd_sb[:], wkvd[:, :].rearrange("p (k r) -> p k r", k=DK)
        )
        wkup_sb = const.tile([128, CK, 128], BF16, tag="wkup")
        nc.sync.dma_start(
            wkup_sb[:], wkup[:, :].rearrange("p (k c) -> p k c", k=CK)
        )
        wkupP_sb = const.tile([128, CK, 128], BF16, tag="wkupP")
        nc.sync.dma_start(
            wkupP_sb[:], wkupP[:, :].rearrange("p (k c) -> p k c", k=CK)
        )
        wvup_sb = const.tile([128, 2, 512], BF16, tag="wvup")
        nc.sync.dma_start(
            wvup_sb[:], wvup[:, :].rearrange("p (k c) -> p k c", k=2)
        )
        cosk_sb = const.tile([128, KEYS], F32, tag="cosk")
        nc.sync.dma_start(cosk_sb[:], cosk[:, :])
        sink_sb = const.tile([128, KEYS], F32, tag="sink")
        nc.sync.dma_start(sink_sb[:], sink[:, :])
        cosq_sb = const.tile([128, SQ], F32, tag="cosq")
        nc.sync.dma_start(cosq_sb[:], cosq[:, :])
        sinq_sb = const.tile([128, SQ], F32, tag="sinq")
        nc.sync.dma_start(sinq_sb[:], sinq[:, :])
        mask_sb = const.tile([128, 2, SQ], BF16, tag="mask")
        nc.sync.dma_start(
            mask_sb[:], maskT[:, :].rearrange("p (m s) -> p m s", m=2)
        )
        wout_sb = const.tile([128, CK, D], BF16, tag="wout")
        nc.sync.dma_start(
            wout_sb[:], wout[:, :].rearrange("p (k c) -> p k c", k=CK)
        )

        # ---- persistent intermediates ----
        lat_sb = const.tile([128, KEYS], BF16, tag="lat")
        kT_sb = const.tile([128, CK, KEYS], BF16, tag="kT")
        # v_sb[:, mk, h, 0:64] = v head h (keys chunk mk); [.., 64:128] = ones
        v_sb = const.tile([128, 2, H, 128], BF16, tag="v")
        qTr_sb = const.tile([128, CK, SQ], BF16, tag="qTr")
        yT_sb = const.tile([128, CK, SQ], BF16, tag="yT")
        warm_lhs = const.tile([128, 128], BF16, tag="wl")
        warm_rhs = const.tile([128, 512], BF16, tag="wr")

        nc.gpsimd.memset(warm_lhs[:], 0.0)
        nc.gpsimd.memset(warm_rhs[:], 0.0)
        nc.gpsimd.memset(v_sb[:], 0.0)
        nc.gpsimd.memset(v_sb[:, :, :, 64:128], 1.0)

        with (
            tc.tile_pool(name="ps1", bufs=6, space="PSUM") as ps1,
            tc.tile_pool(name="qraw_pool", bufs=3) as qraw_pool,
            tc.tile_pool(name="scr", bufs=4) as scr,
        ):
            # ---- PE warmup: ramp the tensor-engine clock while DMAs land
            wps = ps1.tile([128, SQ], F32, tag="b")
            for i in range(N_WARMUP):
                nc.tensor.matmul(
                    wps[:],
                    warm_lhs[:],
                    warm_rhs[:],
                    start=(i == 0),
                    stop=(i == N_WARMUP - 1),
                )

            # ---- stage D: qT chunks (ck-outer, dk accumulation) + RoPE
            for ck in range(CK):
                q_ps = ps1.tile([128, SQ], F32, tag="b")
                for dk in range(DK):
                    nc.tensor.matmul(
                        q_ps[:],
                        wq_sb[ck][:, dk, :],
                        xT_sb[:, dk, :],
                        start=(dk == 0),
                        stop=(dk == DK - 1),
                    )
                qraw = qraw_pool.tile([128, SQ], F32R, tag="qraw")
                nc.scalar.copy(qraw[:], q_ps[:])
                rot_ps = ps1.tile([128, SQ], F32, tag="b")
                nc.tensor.matmul(
                    rot_ps[:], perm_sb[:], qraw[:], start=True, stop=True
                )
                qt1 = scr.tile([128, SQ], F32, tag="qt1")
                nc.gpsimd.tensor_mul(qt1[:], qraw[:], cosq_sb[:])
                qt2 = scr.tile([128, SQ], F32, tag="qt2")
                nc.vector.tensor_mul(qt2[:], rot_ps[:], sinq_sb[:])
                nc.gpsimd.tensor_add(qTr_sb[:, ck, :], qt1[:], qt2[:])

            # ---- stage A: latent at selected positions [R, KEYS]
            lat_full = ps1.tile([128, SQ], F32, tag="b")
            lat_ps = lat_full[:, :KEYS]
            for dk in range(DK):
                nc.tensor.matmul(
                    lat_ps,
                    wkvd_sb[:, dk, :],
                    xsel_sb[:, dk, :],
                    start=(dk == 0),
                    stop=(dk == DK - 1),
                )
            nc.scalar.copy(lat_sb[:], lat_ps)

            # ---- stage B: kT + RoPE (rotation via wkupP weights)
            for ck in range(CK):
                kraw_full = ps1.tile([128, SQ], F32, tag="b")
                kraw_ps = kraw_full[:, :KEYS]
                nc.tensor.matmul(
                    kraw_ps, wkup_sb[:, ck, :], lat_sb[:], start=True, stop=True
                )
                krot_full = ps1.tile([128, SQ], F32, tag="b")
                krot_ps = krot_full[:, :KEYS]
                nc.tensor.matmul(
                    krot_ps, wkupP_sb[:, ck, :], lat_sb[:], start=True, stop=True
                )
                kt1 = scr.tile([128, KEYS], F32, tag="kt1")
                nc.vector.tensor_mul(kt1[:], kraw_ps, cosk_sb[:])
                kt2 = scr.tile([128, KEYS], F32, tag="kt2")
                nc.vector.tensor_mul(kt2[:], krot_ps, sink_sb[:])
                nc.gpsimd.tensor_add(kT_sb[:, ck, :], kt1[:], kt2[:])

            # ---- stage C: v [keys, head-major dims] -> v_sb packed
            for mk in range(2):
                for half in range(2):
                    v_ps = ps1.tile([128, 512], F32, tag="b")
                    nc.tensor.matmul(
                        v_ps[:],
                        lat_sb[:, mk * 128 : (mk + 1) * 128],
                        wvup_sb[:, half, :],
                        start=True,
                        stop=True,
                    )
                    # strided copy: head h cols h*64..h*64+64 -> v_sb[:, mk, h, 0:64]
                    dst = v_sb[:, mk, half * 8 : (half + 1) * 8, 0:64]
                    src = v_ps[:].rearrange("p (h c) -> p h c", h=8)
                    if half == 0:
                        nc.scalar.copy(dst, src)
                    else:
                        nc.vector.tensor_copy(dst, src)

        # ================= stage E: attention per head ====================
        with (
            tc.tile_pool(name="ps_sc", bufs=3, space="PSUM") as ps_sc,
            tc.tile_pool(name="ps_pv", bufs=2, space="PSUM") as ps_pv,
            tc.tile_pool(name="epool", bufs=3) as epool,
            tc.tile_pool(name="ework", bufs=3) as ework,
        ):
            for h in range(H):
                p, hi = divmod(h, 2)
                pb = hi * 64
                sc_ps = ps_sc.tile([128, 2, SQ], F32, tag="sc")
                for mk in range(2):
                    nc.tensor.matmul(
                        sc_ps[:, mk, :],
                        kT_sb[pb : pb + 64, p, mk * 128 : (mk + 1) * 128],
                        qTr_sb[pb : pb + 64, p, :],
                        start=True,
                        stop=True,
                    )
                expU = epool.tile([128, 2, SQ], BF16, tag="expU")
                nc.scalar.activation(
                    expU[:].rearrange("p m s -> p (m s)"),
                    sc_ps[:].rearrange("p m s -> p (m s)"),
                    EXP,
                    scale=0.125,
                )
                expT = epool.tile([128, 2, SQ], BF16, tag="expT")
                eng = nc.gpsimd if hi == 0 else nc.vector
                eng.tensor_mul(
                    expT[:].rearrange("p m s -> p (m s)"),
                    expU[:].rearrange("p m s -> p (m s)"),
                    mask_sb[:].rearrange("p m s -> p (m s)"),
                )
                pv_ps = ps_pv.tile([128, SQ], F32, tag="pv")
                for mk in range(2):
                    nc.tensor.matmul(
                        pv_ps[:],
                        v_sb[:, mk, h, :],
                        expT[:, mk, :],
                        start=(mk == 0),
                        stop=(mk == 1),
                    )
                zm = ework.tile([64, SQ], F32, tag="zm")
                nc.vector.tensor_scalar_max(zm[:], pv_ps[64:128, :], 1e-30)
                zr = ework.tile([64, SQ], F32, tag="zr")
                nc.vector.reciprocal_approx_fast(zr[:], zm[:])
                nc.vector.tensor_mul(
                    yT_sb[pb : pb + 64, p, :], pv_ps[0:64, :], zr[:]
                )

        # ================= stage F: out = yT.T @ wout (st-outer) ==========
        with (
            tc.tile_pool(name="ps_w", bufs=2, space="PSUM") as ps_w,
            tc.tile_pool(name="ost", bufs=2) as ost,
        ):
            for st in range(4):
                o_sb = ost.tile([128, D], F32, tag="osb")
                for dh in range(2):
                    o_ps = ps_w.tile([128, 512], F32, tag="ops")
                    for ck in range(CK):
                        nc.tensor.matmul(
                            o_ps[:],
                            yT_sb[:, ck, st * 128 : (st + 1) * 128],
                            wout_sb[:, ck, dh * 512 : (dh + 1) * 512],
                            start=(ck == 0),
                            stop=(ck == CK - 1),
                        )
                    if dh == 0:
                        nc.scalar.copy(o_sb[:, 0:512], o_ps[:])
                    else:
                        nc.vector.tensor_copy(o_sb[:, 512:1024], o_ps[:])
                nc.sync.dma_start(out[st * 128 : (st + 1) * 128, :], o_sb[:])

    nc.compile()
    return nc


_NC_CACHE = {}


def _get_nc():
    if "nc" not in _NC_CACHE:
        _NC_CACHE["nc"] = build_nc()
    return _NC_CACHE["nc"]


TRACE = False  # set by test harness to capture an NTFF profile
LAST_RESULTS = None


def kernel(x, w_q, w_kv_down, w_kv_up, w_out, w_scorer):
    global LAST_RESULTS
    from concourse.bass_utils import run_bass_kernel_spmd

    in_maps, qmin, fb_rows = host_prep(x, w_q, w_kv_down, w_kv_up, w_out, w_scorer)
    nc = _get_nc()
    res = run_bass_kernel_spmd(nc, in_maps, core_ids=list(range(8)), trace=TRACE)
    LAST_RESULTS = res
    out = np.empty((B, S, D), np.float32)
    for c in range(8):
        b, sq = divmod(c, 4)
        out[b, sq * SQ : (sq + 1) * SQ] = res.results[c]["out"]
    for b in range(B):
        if qmin[b] > 0:
            out[b, : qmin[b]] = fb_rows[b]
    return out
